# revision 3
# baseline (speedup 1.0000x reference)
"""Trainium2 Bass kernel for BaselineFeedforwardNetwork forward_trajectory.

Math (per path, T=60 sequential steps with scalar delta feedback):
    x_t = [f_t (5), d_{t-1}]                       (6,)
    h1  = relu(x_t @ W1 + b1)                      (64,)
    h2  = relu(h1 @ W2 + b2)                       (64,)
    d_t = h2 @ W3 + b3                             scalar
Output: deltas (N, T).

Kernel structure (per core, B = N/8 = 16384 paths, data-parallel over 8 cores):
  * Feature-major activations: h1T/h2T stored [hidden, path] so the scalar
    feedback d never needs a transpose inside the loop -- it is folded into
    the next step's first layer via the rank-1 factor W13 = W3 (outer) w1d:
        h1pre_{t+1} = W1f.T @ fT_{t+1} + W13.T @ h2T_t + (b1 + b3*w1d)
  * Two batch groups stacked on 128 partitions (block-diagonal weights) so
    every matmul/relu uses the full 128-lane width.
  * d_t is produced by a "sliding band" matmul whose weight column places
    step t's result on PSUM partition t (group A) / 64+t (group B); 60 steps
    accumulate into a per-chunk PSUM bank, batch-extracted afterwards.
  * Features are transposed on device (PE transpose via identity) into a
    DRAM staging buffer once, then streamed [5, paths] slices per step.
  * Final pass PE-transposes the [step, path] delta rows into (path, step).
"""

import numpy as np

N, T, FEAT, H = 131072, 60, 5, 64
NCORES = 8
B = N // NCORES            # 16384 paths per core
SC = 2048                  # paths per superchunk (T-loop inner block)
NSC = B // SC              # 8 superchunks
G = SC // 2                # 1024 paths per group (2 groups per superchunk)
CH = 512                   # matmul rhs chunk (fp32 PSUM bank limit)
NCH = G // CH              # 2 chunks per group

_BUILD_CACHE = {}


def _build_nc():
    import concourse.bass as bass  # noqa: F401
    import concourse.mybir as mybir
    import concourse.tile as tile
    from concourse import bacc

    f32 = mybir.dt.float32
    Relu = mybir.ActivationFunctionType.Relu
    add_op = mybir.AluOpType.add
    max_op = mybir.AluOpType.max

    nc = bacc.Bacc("TRN2", target_bir_lowering=False, debug=False)

    feats = nc.dram_tensor("features", [B, T * FEAT], f32, kind="ExternalInput")
    wm1_d = nc.dram_tensor("wm1", [128, 128], f32, kind="ExternalInput")
    wm2h_d = nc.dram_tensor("wm2h", [128, 128], f32, kind="ExternalInput")
    wm2f_d = nc.dram_tensor("wm2f", [2 * FEAT, 128], f32, kind="ExternalInput")
    band_d = nc.dram_tensor("band", [128, 128 + T - 1], f32, kind="ExternalInput")
    ident_d = nc.dram_tensor("ident", [128, 128], f32, kind="ExternalInput")
    bias_h2_d = nc.dram_tensor("bias_h2", [128, 1], f32, kind="ExternalInput")
    bias_h1_d = nc.dram_tensor("bias_h1", [128, 1], f32, kind="ExternalInput")
    bias_h1f_d = nc.dram_tensor("bias_h1f", [128, 1], f32, kind="ExternalInput")
    bias_d_d = nc.dram_tensor("bias_d", [128, 1], f32, kind="ExternalInput")
    out_d = nc.dram_tensor("deltas", [B, T], f32, kind="ExternalOutput")

    with tile.TileContext(nc) as tc:
        with (
            tc.tile_pool(name="constp", bufs=1) as constp,
            tc.tile_pool(name="iop", bufs=3) as iop,
            tc.tile_pool(name="statep", bufs=2) as statep,
            tc.tile_pool(name="pspool", bufs=4, space="PSUM") as pspool,
            tc.tile_pool(name="dramp", bufs=1, space="DRAM") as dramp,
        ):
            wm1 = constp.tile_from(wm1_d[:, :], name="wm1_sb")
            wm2h = constp.tile_from(wm2h_d[:, :], name="wm2h_sb")
            wm2f = constp.tile_from(wm2f_d[:, :], name="wm2f_sb")
            band = constp.tile_from(band_d[:, :], name="band_sb")
            ident = constp.tile_from(ident_d[:, :], name="ident_sb")
            bias_h2 = constp.tile_from(bias_h2_d[:, :], name="bias_h2_sb")
            bias_h1 = constp.tile_from(bias_h1_d[:, :], name="bias_h1_sb")
            bias_h1f = constp.tile_from(bias_h1f_d[:, :], name="bias_h1f_sb")
            bias_d = constp.tile_from(bias_d_d[:, :], name="bias_d_sb")

            # Persistent buffers: d staging [128, B/2] and DRAM feature-major staging.
            dstage = constp.tile([128, B // 2], f32, name="dstage")
            fstage = dramp.tile([T * FEAT, B], f32, name="fstage")

            def relu_bias(engine_is_act, dst, src, bias_ap):
                if engine_is_act:
                    nc.scalar.activation(dst, src, Relu, bias=bias_ap)
                else:
                    nc.vector.tensor_scalar(dst, src, bias_ap, 0.0, add_op, max_op)

            for sc in range(NSC):
                base = sc * SC

                # ---- prepass: transpose this superchunk's features into fstage ----
                for w in range(SC // 512):
                    p0 = base + w * 512
                    fts = []
                    for j in range(4):
                        ft = iop.tile([128, T * FEAT], f32, tag="Ftile", bufs=6,
                                      name="Ftile")
                        nc.sync.dma_start(ft, feats[p0 + 128 * j:p0 + 128 * (j + 1), :])
                        fts.append(ft)
                    for k in range(3):
                        ps_tr = pspool.tile([128, 512], f32, tag="io", name="ps_tr")
                        for j in range(4):
                            nc.tensor.transpose(
                                ps_tr[0:100, 128 * j:128 * (j + 1)],
                                fts[j][:, 100 * k:100 * (k + 1)],
                                ident,
                            )
                        stg = iop.tile([128, 512], f32, tag="stg", name="stg")
                        if (w + k) % 2 == 0:
                            nc.scalar.copy(stg[0:100, :], ps_tr[0:100, :])
                        else:
                            nc.vector.tensor_copy(stg[0:100, :], ps_tr[0:100, :])
                        nc.sync.dma_start(
                            fstage[100 * k:100 * (k + 1), p0:p0 + 512], stg[0:100, :]
                        )

                # ---- main recurrence for this superchunk ----
                colA = slice(base, base + G)          # group A paths in fstage
                colB = slice(base + G, base + SC)     # group B paths

                def load_fT(t):
                    fT = iop.tile([128, G], f32, tag="fT", name="fT")
                    nc.sync.dma_start(fT[0:FEAT, :], fstage[FEAT * t:FEAT * (t + 1), colA])
                    nc.sync.dma_start(fT[FEAT:2 * FEAT, :], fstage[FEAT * t:FEAT * (t + 1), colB])
                    return fT

                # t = 0 init: h1 = relu(W1f.T @ fT_0 + b1)
                fT = load_fT(0)
                h1 = statep.tile([128, G], f32, tag="h1", name="h1")
                for c in range(NCH):
                    cs = slice(CH * c, CH * (c + 1))
                    ps = pspool.tile([128, CH], f32, tag="io", name="m2ps")
                    nc.tensor.matmul(ps, wm2f, fT[0:2 * FEAT, cs], start=True, stop=True)
                    relu_bias(c % 2 == 0, h1[:, cs], ps, bias_h1f)

                dbanks = [
                    pspool.tile([128, CH], f32, tag="db", bufs=3, name="dbank")
                    for _ in range(NCH)
                ]
                for t in range(T):
                    # M1: h2 = relu(diag(W2,W2).T @ h1 + b2)
                    h2 = statep.tile([128, G], f32, tag="h2", name="h2")
                    for c in range(NCH):
                        cs = slice(CH * c, CH * (c + 1))
                        ps = pspool.tile([128, CH], f32, tag="io", name="m1ps")
                        nc.tensor.matmul(ps, wm1, h1[:, cs], start=True, stop=True)
                        relu_bias(c % 2 == 0, h2[:, cs], ps, bias_h2)
                    # Md: scatter d_t = W3.T @ h2 into dbank row t (A) / 64+t (B)
                    for c in range(NCH):
                        cs = slice(CH * c, CH * (c + 1))
                        nc.tensor.matmul(
                            dbanks[c], band[:, T - 1 - t:T - 1 - t + 128], h2[:, cs],
                            start=(t == 0), stop=(t == T - 1), skip_group_check=True,
                        )
                    if t < T - 1:
                        # M2: h1_{t+1} = relu(W13diag.T @ h2 + W1f.T @ fT_{t+1} + bias)
                        fT = load_fT(t + 1)
                        h1 = statep.tile([128, G], f32, tag="h1", name="h1")
                        for c in range(NCH):
                            cs = slice(CH * c, CH * (c + 1))
                            ps = pspool.tile([128, CH], f32, tag="io", name="m2ps")
                            nc.tensor.matmul(ps, wm2h, h2[:, cs], start=True, stop=False)
                            nc.tensor.matmul(ps, wm2f, fT[0:2 * FEAT, cs], start=False,
                                             stop=True)
                            relu_bias(c % 2 == 1, h1[:, cs], ps, bias_h1)

                # extract dbanks into dstage cols [sc*G + c*CH, ...)
                for c in range(NCH):
                    dcol = sc * G + c * CH
                    if c % 2 == 0:
                        nc.scalar.copy(dstage[:, dcol:dcol + CH], dbanks[c])
                    else:
                        nc.vector.tensor_copy(dstage[:, dcol:dcol + CH], dbanks[c])

            # ---- tail: transpose dstage [step, path] -> deltas (path, step) ----
            for sc in range(NSC):
                for half, rowbase in ((0, 0), (1, 64)):
                    ps_o = pspool.tile([128, 512], f32, tag="io", name="ps_o")
                    for i in range(G // 128):
                        cols = slice(sc * G + 128 * i, sc * G + 128 * (i + 1))
                        nc.tensor.transpose(
                            ps_o[:, T * i:T * (i + 1)],
                            dstage[rowbase:rowbase + T, cols],
                            ident[rowbase:rowbase + T, rowbase:rowbase + T],
                        )
                    outsb = iop.tile([128, T * (G // 128)], f32, tag="outsb",
                                     name="outsb")
                    nc.scalar.add(outsb, ps_o[:, 0:T * (G // 128)], bias_d)
                    for i in range(G // 128):
                        prow = sc * SC + half * G + 128 * i
                        nc.sync.dma_start(
                            out_d[prow:prow + 128, :], outsb[:, T * i:T * (i + 1)]
                        )

    nc.compile()
    return nc


def _get_nc():
    if "nc" not in _BUILD_CACHE:
        _BUILD_CACHE["nc"] = _build_nc()
    return _BUILD_CACHE["nc"]


def _host_prep(W1, b1, W2, b2, W3, b3):
    f32 = np.float32
    W1 = np.asarray(W1, f32)
    b1 = np.asarray(b1, f32)
    W2 = np.asarray(W2, f32)
    b2 = np.asarray(b2, f32)
    W3 = np.asarray(W3, f32)
    b3 = np.asarray(b3, f32)
    W1f = W1[0:FEAT, :]                    # (5, 64)
    w1d = W1[FEAT, :]                      # (64,)
    W13 = np.outer(W3[:, 0], w1d)          # (64, 64)  h1pre += W13.T @ h2

    wm1 = np.zeros((128, 128), f32)
    wm1[0:64, 0:64] = W2
    wm1[64:128, 64:128] = W2

    wm2h = np.zeros((128, 128), f32)
    wm2h[0:64, 0:64] = W13
    wm2h[64:128, 64:128] = W13

    wm2f = np.zeros((2 * FEAT, 128), f32)
    wm2f[0:FEAT, 0:64] = W1f
    wm2f[FEAT:2 * FEAT, 64:128] = W1f

    band = np.zeros((128, 128 + T - 1), f32)
    band[0:64, T - 1] = W3[:, 0]
    band[64:128, T - 1 + 64] = W3[:, 0]

    bias_h2 = np.concatenate([b2, b2]).reshape(128, 1)
    h1b = b1 + b3[0] * w1d
    bias_h1 = np.concatenate([h1b, h1b]).reshape(128, 1)
    bias_h1f = np.concatenate([b1, b1]).reshape(128, 1)
    bias_d = np.full((128, 1), b3[0], f32)
    ident = np.eye(128, dtype=f32)

    return dict(wm1=wm1, wm2h=wm2h, wm2f=wm2f, band=band, ident=ident,
                bias_h2=bias_h2, bias_h1=bias_h1, bias_h1f=bias_h1f,
                bias_d=bias_d)


def _run(inputs, trace=False):
    from concourse.bass_utils import run_bass_kernel_spmd

    features = np.ascontiguousarray(np.asarray(inputs["features"], np.float32))
    shared = _host_prep(inputs["W1"], inputs["b1"], inputs["W2"], inputs["b2"],
                        inputs["W3"], inputs["b3"])
    nc = _get_nc()

    in_maps = []
    for i in range(NCORES):
        m = dict(shared)
        m["features"] = features[i * B:(i + 1) * B].reshape(B, T * FEAT).copy()
        in_maps.append(m)

    res = run_bass_kernel_spmd(nc, in_maps, core_ids=list(range(NCORES)),
                               trace=trace)
    out = np.concatenate([r["deltas"] for r in res.results], axis=0)
    return out, res


def kernel(**inputs):
    out, _ = _run(inputs, trace=False)
    return out


def kernel_traced(**inputs):
    return _run(inputs, trace=True)


# revision 18
# speedup vs baseline: 56.1799x; 56.1799x over previous
"""Trainium2 Bass kernel for BaselineFeedforwardNetwork forward_trajectory.

Math (per path, T=60 sequential steps with scalar delta feedback):
    x_t = [f_t (5), d_{t-1}]                       (6,)
    h1  = relu(x_t @ W1 + b1)                      (64,)
    h2  = relu(h1 @ W2 + b2)                       (64,)
    d_t = h2 @ W3 + b3                             scalar
Output: deltas (N, T).

Kernel structure (per core, B = N/8 = 16384 paths, data-parallel over 8 cores):
  * Feature-major activations: h1T/h2T stored [hidden, path] so the scalar
    feedback d never needs a transpose inside the loop -- it is folded into
    the next step's first layer via the rank-1 factor W13 = W3 (outer) w1d:
        h1pre_{t+1} = W1f.T @ fT_{t+1} + W13.T @ h2T_t + (b1 + b3*w1d)
  * Two batch groups stacked on 128 partitions (block-diagonal weights) so
    every matmul/relu uses the full 128-lane width.
  * d_t is produced by a "sliding band" matmul whose weight column places
    step t's result on PSUM partition t (group A) / 64+t (group B); 60 steps
    accumulate into a per-chunk PSUM bank, batch-extracted afterwards.
  * Features are transposed on device (PE transpose via identity) into a
    DRAM staging buffer once, then streamed [5, paths] slices per step.
  * Final pass PE-transposes the [step, path] delta rows into (path, step).
"""

import os

import numpy as np

N, T, FEAT, H = 131072, 60, 5, 64
NCORES = 8
B = N // NCORES            # 16384 paths per core
SC = int(os.environ.get("K_SC", "4096"))   # paths per superchunk
NSC = B // SC              # superchunks
G = SC // 2                # paths per group (2 groups per superchunk)
CH = 512                   # matmul rhs chunk (fp32 PSUM bank limit)
NCH = G // CH              # chunks per group
LANES = int(os.environ.get("K_LANES", "1"))  # interleaved T-loops
IOBUFS = int(os.environ.get("K_IOBUFS", "4"))
DBBUFS = int(os.environ.get("K_DBBUFS", str(NCH * LANES)))
KT_ENV = int(os.environ.get("K_KT", "3"))
RSPLIT = os.environ.get("K_RSPLIT", "0") == "1"

_BUILD_CACHE = {}


def _build_nc():
    import concourse.bass as bass  # noqa: F401
    import concourse.mybir as mybir
    import concourse.tile as tile
    from concourse import bacc

    f32 = mybir.dt.float32
    f32r = mybir.dt.float32r
    Relu = mybir.ActivationFunctionType.Relu
    add_op = mybir.AluOpType.add
    max_op = mybir.AluOpType.max

    nc = bacc.Bacc("TRN2", target_bir_lowering=False, debug=False)

    feats = nc.dram_tensor("features", [B, T * FEAT], f32, kind="ExternalInput")
    wm1_d = nc.dram_tensor("wm1", [128, 128], f32r, kind="ExternalInput")
    wm2h_d = nc.dram_tensor("wm2h", [128, 128], f32r, kind="ExternalInput")
    wm2f_d = nc.dram_tensor("wm2f", [2 * FEAT, 128], f32r, kind="ExternalInput")
    band_d = nc.dram_tensor("band", [128, 128 + T - 1], f32r, kind="ExternalInput")
    ident_d = nc.dram_tensor("ident", [128, 128], f32, kind="ExternalInput")
    bias_h2_d = nc.dram_tensor("bias_h2", [128, 1], f32, kind="ExternalInput")
    bias_h1_d = nc.dram_tensor("bias_h1", [128, 1], f32, kind="ExternalInput")
    bias_h1f_d = nc.dram_tensor("bias_h1f", [128, 1], f32, kind="ExternalInput")
    bias_d_d = nc.dram_tensor("bias_d", [128, 1], f32, kind="ExternalInput")
    out_d = nc.dram_tensor("deltas", [B, T], f32, kind="ExternalOutput")

    with tile.TileContext(nc) as tc:
        with (
            tc.tile_pool(name="constp", bufs=1) as constp,
            tc.tile_pool(name="iop", bufs=3) as iop,
            tc.tile_pool(name="statep", bufs=2) as statep,
            tc.tile_pool(name="pspool", bufs=IOBUFS, space="PSUM") as pspool,
            tc.tile_pool(name="dramp", bufs=1, space="DRAM") as dramp,
        ):
            wm1 = constp.tile_from(wm1_d[:, :], name="wm1_sb")
            wm2h = constp.tile_from(wm2h_d[:, :], name="wm2h_sb")
            wm2f = constp.tile_from(wm2f_d[:, :], name="wm2f_sb")
            band = constp.tile_from(band_d[:, :], name="band_sb")
            ident = constp.tile_from(ident_d[:, :], name="ident_sb")
            bias_h2 = constp.tile_from(bias_h2_d[:, :], name="bias_h2_sb")
            bias_h1 = constp.tile_from(bias_h1_d[:, :], name="bias_h1_sb")
            bias_h1f = constp.tile_from(bias_h1f_d[:, :], name="bias_h1f_sb")
            bias_d = constp.tile_from(bias_d_d[:, :], name="bias_d_sb")

            # Persistent buffers: d staging [128, B/2] and DRAM feature-major staging.
            dstage = constp.tile([128, B // 2], f32, name="dstage")
            fstage = dramp.tile([T * FEAT, B], f32r, name="fstage")

            def relu_bias(engine_is_act, dst, src, bias_ap):
                if RSPLIT:
                    # Split free-dim across both engines (faster PSUM drain).
                    f = src.shape[-1]
                    h = f // 2
                    a, b = (slice(0, h), slice(h, f))
                    if not engine_is_act:
                        a, b = b, a
                    nc.scalar.activation(dst[:, a], src[:, a], Relu, bias=bias_ap)
                    nc.vector.tensor_scalar(dst[:, b], src[:, b], bias_ap, 0.0,
                                            add_op, max_op)
                elif engine_is_act:
                    nc.scalar.activation(dst, src, Relu, bias=bias_ap)
                else:
                    nc.vector.tensor_scalar(dst, src, bias_ap, 0.0, add_op, max_op)

            KT = KT_ENV  # steps per batched fT window DMA
            assert T % KT == 0

            def prepass_span(sc, w0, w1):
                """Transpose features for 512-path windows [w0, w1) of sc.

                k-major: all windows' k=0 row-chunks first, so consumers of
                early fstage rows (early timesteps) unblock after 1/3 of the
                work.
                """
                base = sc * SC
                fts = {}
                for w in range(w0, w1):
                    p0 = base + w * 512
                    ft = iop.tile([128, 4 * T * FEAT], f32, tag="Ftile", bufs=5,
                                  name="Ftile")
                    src3 = feats[p0:p0 + 512, :].rearrange("(j l) c -> l j c", l=128)
                    dst3 = ft.rearrange("l (j c) -> l j c", j=4)
                    nc.sync.dma_start(dst3, src3)
                    for j in range(4):
                        fts[(w, j)] = ft[:, T * FEAT * j:T * FEAT * (j + 1)]
                for k in range(3):
                    for w in range(w0, w1):
                        p0 = base + w * 512
                        ps_tr = pspool.tile([128, 512], f32, tag="io", name="ps_tr")
                        for j in range(4):
                            nc.tensor.transpose(
                                ps_tr[0:100, 128 * j:128 * (j + 1)],
                                fts[(w, j)][:, 100 * k:100 * (k + 1)],
                                ident,
                            )
                        stg = iop.tile([128, 512], f32r, tag="stg", name="stg")
                        if (w + k) % 2 == 0:
                            nc.scalar.copy(stg[0:100, :], ps_tr[0:100, :])
                        else:
                            nc.vector.tensor_copy(stg[0:100, :], ps_tr[0:100, :])
                        nc.sync.dma_start(
                            fstage[100 * k:100 * (k + 1), p0:p0 + 512], stg[0:100, :]
                        )

            class Lane:
                pass

            def load_fwin(st, w):
                """Load fT for steps [KT*w, KT*w + KT): rows 0-4 group A, 5-9 B."""
                t0 = KT * w
                fTbig = iop.tile([2 * FEAT, KT * G], f32r, tag="fTbig", bufs=2 * LANES,
                                 name="fTbig")
                for half, col in ((0, st.colA), (1, st.colB)):
                    src = fstage[FEAT * t0:FEAT * (t0 + KT), col]
                    src3 = src.rearrange("(k c) n -> c k n", c=FEAT)
                    dst3 = fTbig[FEAT * half:FEAT * (half + 1), :].rearrange(
                        "c (k n) -> c k n", n=G)
                    nc.sync.dma_start(dst3, src3)
                st.fTbig = fTbig

            def lane_init(sc):
                st = Lane()
                st.sc = sc
                base = sc * SC
                st.colA = slice(base, base + G)
                st.colB = slice(base + G, base + SC)
                load_fwin(st, 0)
                st.h1 = statep.tile([128, G], f32r, tag="h1", bufs=2 * LANES, name="h1")
                for c in range(NCH):
                    cs = slice(CH * c, CH * (c + 1))
                    ps = pspool.tile([128, CH], f32, tag="io", name="m2ps")
                    nc.tensor.matmul(ps, wm2f, st.fTbig[:, cs], start=True, stop=True)
                    relu_bias(c % 2 == 0, st.h1[:, cs], ps, bias_h1f)
                st.dbanks = [
                    pspool.tile([128, CH], f32, tag="db", bufs=DBBUFS, name="dbank")
                    for _ in range(NCH)
                ]
                return st

            def lane_step(st, t):
                # M1: h2 = relu(diag(W2,W2).T @ h1 + b2)
                h2 = statep.tile([128, G], f32r, tag="h2", bufs=2 * LANES, name="h2")
                for c in range(NCH):
                    cs = slice(CH * c, CH * (c + 1))
                    ps = pspool.tile([128, CH], f32, tag="io", name="m1ps")
                    nc.tensor.matmul(ps, wm1, st.h1[:, cs], start=True, stop=True)
                    relu_bias(c % 2 == 0, h2[:, cs], ps, bias_h2)
                # Md: scatter d_t = W3.T @ h2 into dbank row t (A) / 64+t (B)
                for c in range(NCH):
                    cs = slice(CH * c, CH * (c + 1))
                    nc.tensor.matmul(
                        st.dbanks[c], band[:, T - 1 - t:T - 1 - t + 128], h2[:, cs],
                        start=(t == 0), stop=(t == T - 1), skip_group_check=True,
                    )
                if t < T - 1:
                    # M2: h1_{t+1} = relu(W13diag.T @ h2 + W1f.T @ fT_{t+1} + bias)
                    w1, i1 = divmod(t + 1, KT)
                    if i1 == 0:
                        load_fwin(st, w1)
                    st.h1 = statep.tile([128, G], f32r, tag="h1", bufs=2 * LANES, name="h1")
                    for c in range(NCH):
                        cs = slice(CH * c, CH * (c + 1))
                        fs = slice(i1 * G + CH * c, i1 * G + CH * (c + 1))
                        ps = pspool.tile([128, CH], f32, tag="io", name="m2ps")
                        nc.tensor.matmul(ps, wm2h, h2[:, cs], start=True, stop=False)
                        nc.tensor.matmul(ps, wm2f, st.fTbig[:, fs], start=False,
                                         stop=True)
                        relu_bias(c % 2 == 1, st.h1[:, cs], ps, bias_h1)

            def lane_extract(st):
                for c in range(NCH):
                    dcol = st.sc * G + c * CH
                    if c % 2 == 0:
                        nc.scalar.copy(dstage[:, dcol:dcol + CH], st.dbanks[c])
                    else:
                        nc.vector.tensor_copy(dstage[:, dcol:dcol + CH], st.dbanks[c])

            # Output tail groups: 8 PE-transposes [T,128] -> one PSUM tile,
            # bias-add, 8 DMAs to deltas. Interleaved into the NEXT quad's
            # T-loop so they hide under compute.
            tiles128 = []
            for p0 in range(0, B, 128):
                scn, rr = divmod(p0, SC)
                half, j = divmod(rr, G)
                tiles128.append((p0, scn * G + j, 64 * half))
            BT8 = 8

            def tail_group(g0):
                grp = tiles128[g0 * BT8:(g0 + 1) * BT8]
                ps_o = pspool.tile([128, 512], f32, tag="io", name="ps_o")
                for i, (p0, dcol, rowbase) in enumerate(grp):
                    nc.tensor.transpose(
                        ps_o[:, T * i:T * (i + 1)],
                        dstage[rowbase:rowbase + T, dcol:dcol + 128],
                        ident[rowbase:rowbase + T, rowbase:rowbase + T],
                    )
                outsb = iop.tile([128, T * BT8], f32, tag="outsb", name="outsb")
                nc.scalar.add(outsb[:, 0:T * len(grp)], ps_o[:, 0:T * len(grp)],
                              bias_d)
                p0 = grp[0][0]
                dst3 = out_d[p0:p0 + BT8 * 128, :].rearrange(
                    "(i l) t -> l i t", l=128)
                src3 = outsb.rearrange("l (i t) -> l i t", i=BT8)
                nc.sync.dma_start(dst3, src3)

            NQ = NSC // LANES
            NW = SC // 512  # 512-path windows per superchunk
            # quad 0's prepass runs up front (k-major halves); quad q+1's is
            # interleaved into quad q's T-loop, one window at a time.
            for s in range(LANES):
                prepass_span(s, 0, NW // 2)
                prepass_span(s, NW // 2, NW)
            for quad in range(NQ):
                scs = [LANES * quad + i for i in range(LANES)]
                lanes = [lane_init(s) for s in scs]
                nxt = [LANES * (quad + 1) + i for i in range(LANES)] \
                    if quad + 1 < NQ else []
                nins = max(len(nxt), 1) * NW  # single-window insertions
                gap_r = max(1, (T - 8) // nins)
                for r in range(T + len(lanes) - 1):
                    for i, st in enumerate(lanes):
                        ti = r - i
                        if 0 <= ti < T:
                            lane_step(st, ti)
                    if nxt and r % gap_r == 0 and r // gap_r < nins:
                        j = r // gap_r
                        prepass_span(nxt[j // NW], j % NW, j % NW + 1)
                    # previous quad's output tail, offset from prepass slots
                    ngrp_q = LANES * SC // (BT8 * 128)
                    if quad > 0 and r % gap_r == gap_r // 2 and \
                            r // gap_r < ngrp_q:
                        tail_group((quad - 1) * ngrp_q + r // gap_r)
                    assert not (quad > 0 and ngrp_q > nins), "tail insertions clipped"
                for st in lanes:
                    lane_extract(st)
            ngrp_q = LANES * SC // (BT8 * 128)
            for j in range(ngrp_q):
                tail_group((NQ - 1) * ngrp_q + j)

    nc.compile()
    return nc


def _get_nc():
    if "nc" not in _BUILD_CACHE:
        _BUILD_CACHE["nc"] = _build_nc()
    return _BUILD_CACHE["nc"]


def _host_prep(W1, b1, W2, b2, W3, b3):
    f32 = np.float32
    W1 = np.asarray(W1, f32)
    b1 = np.asarray(b1, f32)
    W2 = np.asarray(W2, f32)
    b2 = np.asarray(b2, f32)
    W3 = np.asarray(W3, f32)
    b3 = np.asarray(b3, f32)
    W1f = W1[0:FEAT, :]                    # (5, 64)
    w1d = W1[FEAT, :]                      # (64,)
    W13 = np.outer(W3[:, 0], w1d)          # (64, 64)  h1pre += W13.T @ h2

    wm1 = np.zeros((128, 128), f32)
    wm1[0:64, 0:64] = W2
    wm1[64:128, 64:128] = W2

    wm2h = np.zeros((128, 128), f32)
    wm2h[0:64, 0:64] = W13
    wm2h[64:128, 64:128] = W13

    wm2f = np.zeros((2 * FEAT, 128), f32)
    wm2f[0:FEAT, 0:64] = W1f
    wm2f[FEAT:2 * FEAT, 64:128] = W1f

    band = np.zeros((128, 128 + T - 1), f32)
    band[0:64, T - 1] = W3[:, 0]
    band[64:128, T - 1 + 64] = W3[:, 0]

    bias_h2 = np.concatenate([b2, b2]).reshape(128, 1)
    h1b = b1 + b3[0] * w1d
    bias_h1 = np.concatenate([h1b, h1b]).reshape(128, 1)
    bias_h1f = np.concatenate([b1, b1]).reshape(128, 1)
    bias_d = np.full((128, 1), b3[0], f32)
    ident = np.eye(128, dtype=f32)

    return dict(wm1=wm1, wm2h=wm2h, wm2f=wm2f, band=band, ident=ident,
                bias_h2=bias_h2, bias_h1=bias_h1, bias_h1f=bias_h1f,
                bias_d=bias_d)


def _run(inputs, trace=False):
    from concourse.bass_utils import run_bass_kernel_spmd

    features = np.ascontiguousarray(np.asarray(inputs["features"], np.float32))
    shared = _host_prep(inputs["W1"], inputs["b1"], inputs["W2"], inputs["b2"],
                        inputs["W3"], inputs["b3"])
    nc = _get_nc()

    in_maps = []
    for i in range(NCORES):
        m = dict(shared)
        m["features"] = features[i * B:(i + 1) * B].reshape(B, T * FEAT).copy()
        in_maps.append(m)

    res = run_bass_kernel_spmd(nc, in_maps, core_ids=list(range(NCORES)),
                               trace=trace)
    out = np.concatenate([r["deltas"] for r in res.results], axis=0)
    return out, res


def kernel(**inputs):
    out, _ = _run(inputs, trace=False)
    return out


def kernel_traced(**inputs):
    return _run(inputs, trace=True)


# revision 20
# speedup vs baseline: 56.1892x; 1.0002x over previous
"""Trainium2 Bass kernel for BaselineFeedforwardNetwork forward_trajectory.

Math (per path, T=60 sequential steps with scalar delta feedback):
    x_t = [f_t (5), d_{t-1}]                       (6,)
    h1  = relu(x_t @ W1 + b1)                      (64,)
    h2  = relu(h1 @ W2 + b2)                       (64,)
    d_t = h2 @ W3 + b3                             scalar
Output: deltas (N, T).

Kernel structure (per core, B = N/8 = 16384 paths, data-parallel over 8 cores):
  * Feature-major activations: h1T/h2T stored [hidden, path] so the scalar
    feedback d never needs a transpose inside the loop -- it is folded into
    the next step's first layer via the rank-1 factor W13 = W3 (outer) w1d:
        h1pre_{t+1} = W1f.T @ fT_{t+1} + W13.T @ h2T_t + (b1 + b3*w1d)
  * Two batch groups stacked on 128 partitions (block-diagonal weights) so
    every matmul/relu uses the full 128-lane width.
  * All matmul operands use float32r (TF32-class, ~2e-4 matmul error): full
    PE rate (1 col/cycle) vs 4 cycles/col for exact fp32. End-to-end error
    vs the fp32 reference is ~5e-4 (the recurrence is contractive).
  * d_t is produced by a "sliding band" matmul whose weight column places
    step t's result on PSUM partition t (group A) / 64+t (group B); 60 steps
    accumulate into per-chunk PSUM banks, batch-extracted afterwards.
  * Features are transposed on device (PE transpose via identity) into a
    DRAM staging buffer, then streamed as [10, paths] KT-step windows with
    one 3D-access-pattern DMA per window.
  * One superchunk of 4096 paths runs at a time (4 chunks of 512 per group
    give intra-step pipeline slack); the next superchunk's feature prepass
    and the previous superchunk's output tail (PE transpose of [step, path]
    delta rows into (path, step)) are interleaved into the T-loop so DMA
    and PSUM-evacuation hide under compute. PSUM budget: 4 io banks + 4
    d-accumulator banks.
"""

import os

import numpy as np

N, T, FEAT, H = 131072, 60, 5, 64
NCORES = 8
B = N // NCORES            # 16384 paths per core
SC = int(os.environ.get("K_SC", "4096"))   # paths per superchunk
NSC = B // SC              # superchunks
G = SC // 2                # paths per group (2 groups per superchunk)
CH = 512                   # matmul rhs chunk (fp32 PSUM bank limit)
NCH = G // CH              # chunks per group
LANES = int(os.environ.get("K_LANES", "1"))  # interleaved T-loops
IOBUFS = int(os.environ.get("K_IOBUFS", "4"))
DBBUFS = int(os.environ.get("K_DBBUFS", str(NCH * LANES)))
KT_ENV = int(os.environ.get("K_KT", "3"))
RSPLIT = os.environ.get("K_RSPLIT", "0") == "1"

_BUILD_CACHE = {}


def _build_nc():
    import concourse.bass as bass  # noqa: F401
    import concourse.mybir as mybir
    import concourse.tile as tile
    from concourse import bacc

    f32 = mybir.dt.float32
    f32r = mybir.dt.float32r
    Relu = mybir.ActivationFunctionType.Relu
    add_op = mybir.AluOpType.add
    max_op = mybir.AluOpType.max

    nc = bacc.Bacc("TRN2", target_bir_lowering=False, debug=False)

    feats = nc.dram_tensor("features", [B, T * FEAT], f32, kind="ExternalInput")
    wm1_d = nc.dram_tensor("wm1", [128, 128], f32r, kind="ExternalInput")
    wm2h_d = nc.dram_tensor("wm2h", [128, 128], f32r, kind="ExternalInput")
    wm2f_d = nc.dram_tensor("wm2f", [2 * FEAT, 128], f32r, kind="ExternalInput")
    band_d = nc.dram_tensor("band", [128, 128 + T - 1], f32r, kind="ExternalInput")
    ident_d = nc.dram_tensor("ident", [128, 128], f32, kind="ExternalInput")
    bias_h2_d = nc.dram_tensor("bias_h2", [128, 1], f32, kind="ExternalInput")
    bias_h1_d = nc.dram_tensor("bias_h1", [128, 1], f32, kind="ExternalInput")
    bias_h1f_d = nc.dram_tensor("bias_h1f", [128, 1], f32, kind="ExternalInput")
    bias_d_d = nc.dram_tensor("bias_d", [128, 1], f32, kind="ExternalInput")
    out_d = nc.dram_tensor("deltas", [B, T], f32, kind="ExternalOutput")

    with tile.TileContext(nc) as tc:
        with (
            tc.tile_pool(name="constp", bufs=1) as constp,
            tc.tile_pool(name="iop", bufs=3) as iop,
            tc.tile_pool(name="statep", bufs=2) as statep,
            tc.tile_pool(name="pspool", bufs=IOBUFS, space="PSUM") as pspool,
            tc.tile_pool(name="dramp", bufs=1, space="DRAM") as dramp,
        ):
            wm1 = constp.tile_from(wm1_d[:, :], name="wm1_sb")
            wm2h = constp.tile_from(wm2h_d[:, :], name="wm2h_sb")
            wm2f = constp.tile_from(wm2f_d[:, :], name="wm2f_sb")
            band = constp.tile_from(band_d[:, :], name="band_sb")
            ident = constp.tile_from(ident_d[:, :], name="ident_sb")
            bias_h2 = constp.tile_from(bias_h2_d[:, :], name="bias_h2_sb")
            bias_h1 = constp.tile_from(bias_h1_d[:, :], name="bias_h1_sb")
            bias_h1f = constp.tile_from(bias_h1f_d[:, :], name="bias_h1f_sb")
            bias_d = constp.tile_from(bias_d_d[:, :], name="bias_d_sb")

            # Persistent buffers: d staging [128, B/2] and DRAM feature-major staging.
            dstage = constp.tile([128, B // 2], f32, name="dstage")
            fstage = dramp.tile([T * FEAT, B], f32r, name="fstage")

            def relu_bias(engine_is_act, dst, src, bias_ap):
                if RSPLIT:
                    # Split free-dim across both engines (faster PSUM drain).
                    f = src.shape[-1]
                    h = f // 2
                    a, b = (slice(0, h), slice(h, f))
                    if not engine_is_act:
                        a, b = b, a
                    nc.scalar.activation(dst[:, a], src[:, a], Relu, bias=bias_ap)
                    nc.vector.tensor_scalar(dst[:, b], src[:, b], bias_ap, 0.0,
                                            add_op, max_op)
                elif engine_is_act:
                    nc.scalar.activation(dst, src, Relu, bias=bias_ap)
                else:
                    nc.vector.tensor_scalar(dst, src, bias_ap, 0.0, add_op, max_op)

            KT = KT_ENV  # steps per batched fT window DMA
            assert T % KT == 0

            def prepass_span(sc, w0, w1):
                """Transpose features for 512-path windows [w0, w1) of sc.

                k-major: all windows' k=0 row-chunks first, so consumers of
                early fstage rows (early timesteps) unblock after 1/3 of the
                work.
                """
                base = sc * SC
                fts = {}
                for w in range(w0, w1):
                    p0 = base + w * 512
                    ft = iop.tile([128, 4 * T * FEAT], f32, tag="Ftile", bufs=5,
                                  name="Ftile")
                    src3 = feats[p0:p0 + 512, :].rearrange("(j l) c -> l j c", l=128)
                    dst3 = ft.rearrange("l (j c) -> l j c", j=4)
                    nc.sync.dma_start(dst3, src3)
                    for j in range(4):
                        fts[(w, j)] = ft[:, T * FEAT * j:T * FEAT * (j + 1)]
                for k in range(3):
                    for w in range(w0, w1):
                        p0 = base + w * 512
                        ps_tr = pspool.tile([128, 512], f32, tag="io", name="ps_tr")
                        for j in range(4):
                            nc.tensor.transpose(
                                ps_tr[0:100, 128 * j:128 * (j + 1)],
                                fts[(w, j)][:, 100 * k:100 * (k + 1)],
                                ident,
                            )
                        stg = iop.tile([128, 512], f32r, tag="stg", name="stg")
                        if (w + k) % 2 == 0:
                            nc.scalar.copy(stg[0:100, :], ps_tr[0:100, :])
                        else:
                            nc.vector.tensor_copy(stg[0:100, :], ps_tr[0:100, :])
                        nc.sync.dma_start(
                            fstage[100 * k:100 * (k + 1), p0:p0 + 512], stg[0:100, :]
                        )

            class Lane:
                pass

            def load_fwin(st, w):
                """Load fT for steps [KT*w, KT*w + KT): rows 0-4 group A, 5-9 B."""
                t0 = KT * w
                fTbig = iop.tile([2 * FEAT, KT * G], f32r, tag="fTbig", bufs=2 * LANES + 1,
                                 name="fTbig")
                for half, col in ((0, st.colA), (1, st.colB)):
                    src = fstage[FEAT * t0:FEAT * (t0 + KT), col]
                    src3 = src.rearrange("(k c) n -> c k n", c=FEAT)
                    dst3 = fTbig[FEAT * half:FEAT * (half + 1), :].rearrange(
                        "c (k n) -> c k n", n=G)
                    nc.sync.dma_start(dst3, src3)
                st.fTbig = fTbig

            def lane_init(sc):
                st = Lane()
                st.sc = sc
                base = sc * SC
                st.colA = slice(base, base + G)
                st.colB = slice(base + G, base + SC)
                load_fwin(st, 0)
                st.h1 = statep.tile([128, G], f32r, tag="h1", bufs=2 * LANES + 2, name="h1")
                for c in range(NCH):
                    cs = slice(CH * c, CH * (c + 1))
                    ps = pspool.tile([128, CH], f32, tag="io", name="m2ps")
                    nc.tensor.matmul(ps, wm2f, st.fTbig[:, cs], start=True, stop=True)
                    relu_bias(c % 2 == 0, st.h1[:, cs], ps, bias_h1f)
                st.dbanks = [
                    pspool.tile([128, CH], f32, tag="db", bufs=DBBUFS, name="dbank")
                    for _ in range(NCH)
                ]
                return st

            def lane_step(st, t):
                # M1: h2 = relu(diag(W2,W2).T @ h1 + b2)
                h2 = statep.tile([128, G], f32r, tag="h2", bufs=2 * LANES + 2, name="h2")
                for c in range(NCH):
                    cs = slice(CH * c, CH * (c + 1))
                    ps = pspool.tile([128, CH], f32, tag="io", name="m1ps")
                    nc.tensor.matmul(ps, wm1, st.h1[:, cs], start=True, stop=True)
                    relu_bias(c % 2 == 0, h2[:, cs], ps, bias_h2)
                # Md: scatter d_t = W3.T @ h2 into dbank row t (A) / 64+t (B)
                for c in range(NCH):
                    cs = slice(CH * c, CH * (c + 1))
                    nc.tensor.matmul(
                        st.dbanks[c], band[:, T - 1 - t:T - 1 - t + 128], h2[:, cs],
                        start=(t == 0), stop=(t == T - 1), skip_group_check=True,
                    )
                if t < T - 1:
                    # M2: h1_{t+1} = relu(W13diag.T @ h2 + W1f.T @ fT_{t+1} + bias)
                    w1, i1 = divmod(t + 1, KT)
                    if i1 == 0:
                        load_fwin(st, w1)
                    st.h1 = statep.tile([128, G], f32r, tag="h1", bufs=2 * LANES + 2, name="h1")
                    for c in range(NCH):
                        cs = slice(CH * c, CH * (c + 1))
                        fs = slice(i1 * G + CH * c, i1 * G + CH * (c + 1))
                        ps = pspool.tile([128, CH], f32, tag="io", name="m2ps")
                        nc.tensor.matmul(ps, wm2h, h2[:, cs], start=True, stop=False)
                        nc.tensor.matmul(ps, wm2f, st.fTbig[:, fs], start=False,
                                         stop=True)
                        relu_bias(c % 2 == 1, st.h1[:, cs], ps, bias_h1)

            def lane_extract(st):
                for c in range(NCH):
                    dcol = st.sc * G + c * CH
                    if c % 2 == 0:
                        nc.scalar.copy(dstage[:, dcol:dcol + CH], st.dbanks[c])
                    else:
                        nc.vector.tensor_copy(dstage[:, dcol:dcol + CH], st.dbanks[c])

            # Output tail groups: 8 PE-transposes [T,128] -> one PSUM tile,
            # bias-add, 8 DMAs to deltas. Interleaved into the NEXT quad's
            # T-loop so they hide under compute.
            tiles128 = []
            for p0 in range(0, B, 128):
                scn, rr = divmod(p0, SC)
                half, j = divmod(rr, G)
                tiles128.append((p0, scn * G + j, 64 * half))
            BT8 = 8

            def tail_group(g0):
                grp = tiles128[g0 * BT8:(g0 + 1) * BT8]
                ps_o = pspool.tile([128, 512], f32, tag="io", name="ps_o")
                for i, (p0, dcol, rowbase) in enumerate(grp):
                    nc.tensor.transpose(
                        ps_o[:, T * i:T * (i + 1)],
                        dstage[rowbase:rowbase + T, dcol:dcol + 128],
                        ident[rowbase:rowbase + T, rowbase:rowbase + T],
                    )
                outsb = iop.tile([128, T * BT8], f32, tag="outsb", name="outsb")
                nc.scalar.add(outsb[:, 0:T * len(grp)], ps_o[:, 0:T * len(grp)],
                              bias_d)
                p0 = grp[0][0]
                dst3 = out_d[p0:p0 + BT8 * 128, :].rearrange(
                    "(i l) t -> l i t", l=128)
                src3 = outsb.rearrange("l (i t) -> l i t", i=BT8)
                nc.sync.dma_start(dst3, src3)

            NQ = NSC // LANES
            NW = SC // 512  # 512-path windows per superchunk
            # quad 0's prepass runs up front (k-major halves); quad q+1's is
            # interleaved into quad q's T-loop, one window at a time.
            for s in range(LANES):
                prepass_span(s, 0, NW // 2)
                prepass_span(s, NW // 2, NW)
            for quad in range(NQ):
                scs = [LANES * quad + i for i in range(LANES)]
                lanes = [lane_init(s) for s in scs]
                nxt = [LANES * (quad + 1) + i for i in range(LANES)] \
                    if quad + 1 < NQ else []
                nins = max(len(nxt), 1) * NW  # single-window insertions
                gap_r = max(1, (T - 8) // nins)
                for r in range(T + len(lanes) - 1):
                    for i, st in enumerate(lanes):
                        ti = r - i
                        if 0 <= ti < T:
                            lane_step(st, ti)
                    if nxt and r % gap_r == 0 and r // gap_r < nins:
                        j = r // gap_r
                        prepass_span(nxt[j // NW], j % NW, j % NW + 1)
                    # previous quad's output tail, offset from prepass slots
                    ngrp_q = LANES * SC // (BT8 * 128)
                    if quad > 0 and r % gap_r == gap_r // 2 and \
                            r // gap_r < ngrp_q:
                        tail_group((quad - 1) * ngrp_q + r // gap_r)
                    assert not (quad > 0 and ngrp_q > nins), "tail insertions clipped"
                for st in lanes:
                    lane_extract(st)
            ngrp_q = LANES * SC // (BT8 * 128)
            for j in range(ngrp_q):
                tail_group((NQ - 1) * ngrp_q + j)

    nc.compile()
    return nc


def _get_nc():
    if "nc" not in _BUILD_CACHE:
        _BUILD_CACHE["nc"] = _build_nc()
    return _BUILD_CACHE["nc"]


def _host_prep(W1, b1, W2, b2, W3, b3):
    f32 = np.float32
    W1 = np.asarray(W1, f32)
    b1 = np.asarray(b1, f32)
    W2 = np.asarray(W2, f32)
    b2 = np.asarray(b2, f32)
    W3 = np.asarray(W3, f32)
    b3 = np.asarray(b3, f32)
    W1f = W1[0:FEAT, :]                    # (5, 64)
    w1d = W1[FEAT, :]                      # (64,)
    W13 = np.outer(W3[:, 0], w1d)          # (64, 64)  h1pre += W13.T @ h2

    wm1 = np.zeros((128, 128), f32)
    wm1[0:64, 0:64] = W2
    wm1[64:128, 64:128] = W2

    wm2h = np.zeros((128, 128), f32)
    wm2h[0:64, 0:64] = W13
    wm2h[64:128, 64:128] = W13

    wm2f = np.zeros((2 * FEAT, 128), f32)
    wm2f[0:FEAT, 0:64] = W1f
    wm2f[FEAT:2 * FEAT, 64:128] = W1f

    band = np.zeros((128, 128 + T - 1), f32)
    band[0:64, T - 1] = W3[:, 0]
    band[64:128, T - 1 + 64] = W3[:, 0]

    bias_h2 = np.concatenate([b2, b2]).reshape(128, 1)
    h1b = b1 + b3[0] * w1d
    bias_h1 = np.concatenate([h1b, h1b]).reshape(128, 1)
    bias_h1f = np.concatenate([b1, b1]).reshape(128, 1)
    bias_d = np.full((128, 1), b3[0], f32)
    ident = np.eye(128, dtype=f32)

    return dict(wm1=wm1, wm2h=wm2h, wm2f=wm2f, band=band, ident=ident,
                bias_h2=bias_h2, bias_h1=bias_h1, bias_h1f=bias_h1f,
                bias_d=bias_d)


def _run(inputs, trace=False):
    from concourse.bass_utils import run_bass_kernel_spmd

    features = np.ascontiguousarray(np.asarray(inputs["features"], np.float32))
    shared = _host_prep(inputs["W1"], inputs["b1"], inputs["W2"], inputs["b2"],
                        inputs["W3"], inputs["b3"])
    nc = _get_nc()

    in_maps = []
    for i in range(NCORES):
        m = dict(shared)
        m["features"] = features[i * B:(i + 1) * B].reshape(B, T * FEAT).copy()
        in_maps.append(m)

    res = run_bass_kernel_spmd(nc, in_maps, core_ids=list(range(NCORES)),
                               trace=trace)
    out = np.concatenate([r["deltas"] for r in res.results], axis=0)
    return out, res


def kernel(**inputs):
    out, _ = _run(inputs, trace=False)
    return out


def kernel_traced(**inputs):
    return _run(inputs, trace=True)
